# revision 23
# baseline (speedup 1.0000x reference)
"""Self-attention kernel for Trainium2 (8 NeuronCores, SPMD).

Problem: X[8192,512], Wq,Wk[512,512]:
    Q = X@Wq ; K = X@Wk ; S = softmax(Q K^T / sqrt(512)) ; out = S @ X

Sharding: rows of Q (query blocks of 1024) across 8 cores; K/V (=X)
replicated (host-staged).  Host folds M = Wq Wk^T / sqrt(512) so
S = X M X^T needs a single on-device projection G^T = M^T-chunks @ X^T.

Per-core dataflow (core owns query rows i in [c*1024, (c+1)*1024)):
  warmup: ~36 tiny matmuls on zeros to lift the PE HAM clock-gate +
          exp-table preload while the first DMAs land.
  P:  G^T[e,i] accumulated from w2 chunks x the core's OWN two X^T
      stream blocks (the j-block order is rotated per core on host so
      blocks 0,1 of the stream == this core's query rows; no extra DMA).
  Per i-half h (512 query columns):
    B1: stream X^T blocks (f32r): S^T tile [128 j, 512 i] = 4 accum
        matmuls -> ACT copy PSUM->SBUF st, DVE running max mx[128,512]
    fin: partition-reduce mx via PE transpose + DVE reduce_max ->
        [1,512], broadcast back to b_sb[128,512] via ones outer-product
    B3: st -= b_sb in place (DVE); p = exp(st) (ACT, f16); per c-chunk:
        o_ps[i,512v] += p[:,c].T @ x16 tile and sum_ps[128,c] +=
        p[:,c].T @ ones (1-col matmul, shares the loaded stationary)
    B4: DVE reciprocal on sum_ps [128,4], ACT Copy-with-scale drains
        o_ps -> SBUF, DMA out.
  DMA routing: xt + w2 + out on sync HWDGE, x16 value tiles on gpsimd
  SWDGE so the two streams' trigger queues never block each other.
  xt blocks for half 1 are prefetched during B3 of half 0.
"""
import sys

sys.path.insert(0, "/opt/trn_rl_repo")

import numpy as np
import ml_dtypes

import concourse.bass as bass
import concourse.mybir as mybir
import concourse.tile as tile
from concourse import bacc
from concourse.bass import ts
from concourse.bass_utils import run_bass_kernel_spmd
from concourse.masks import make_identity

F32 = mybir.dt.float32
F32R = mybir.dt.float32r
F16 = mybir.dt.float16
BF16 = mybir.dt.bfloat16
AF = mybir.ActivationFunctionType
ALU = mybir.AluOpType

N = 8192
D = 512
NCORES = 8
MY_N = N // NCORES          # 1024 query rows per core
NJT = N // 128              # 64 j-tiles
NBLK = N // 512             # 16 stream blocks
NIH = MY_N // 512           # 2 i-halves

_NC_CACHE = None


def _build_nc():
    nc = bacc.Bacc(None, target_bir_lowering=False)

    xt = nc.dram_tensor("xt", [128, NBLK, 4, 512], F32R, kind="ExternalInput")
    x16 = nc.dram_tensor("x16", [128, NBLK, 4, 512], BF16, kind="ExternalInput")
    w2 = nc.dram_tensor("w2", [128, 4, 512], F32R, kind="ExternalInput")
    o = nc.dram_tensor("o", [MY_N, D], F32, kind="ExternalOutput")

    with tile.TileContext(nc) as tc:
        with (
            tc.tile_pool(name="pool", bufs=1) as pool,          # persistent
            tc.tile_pool(name="stream", bufs=4) as stream,      # xt blocks
            tc.tile_pool(name="big", bufs=1) as big,            # st region
            tc.tile_pool(name="xs", bufs=3) as xsp,             # x16 tiles
            tc.tile_pool(name="work", bufs=3) as work,          # p tiles
            tc.tile_pool(name="osbp", bufs=2) as osbp,
            tc.tile_pool(name="ps_qk", bufs=3, space="PSUM") as ps_qk,
            tc.tile_pool(name="ps_o", bufs=1, space="PSUM") as ps_o,
            tc.tile_pool(name="ps_sum", bufs=1, space="PSUM") as ps_sum,
        ):
            # ---- constants ----
            ident = pool.tile([128, 128], F32)
            make_identity(nc, ident[:])
            zeros16 = pool.tile([128, 128], BF16)
            nc.vector.memset(zeros16[:], 0.0)
            ones_f32 = pool.tile([128, 4], F32)
            nc.vector.memset(ones_f32[:], 1.0)
            ones_col = pool.tile([128, 1], BF16)    # rhs for 1-col row sums
            nc.vector.tensor_copy(ones_col[:], ones_f32[:, 0:1])
            ones4 = pool.tile([128, 4], BF16)
            nc.vector.tensor_copy(ones4[:], ones_f32[:])
            ones_row_f32 = pool.tile([1, 128], F32)
            nc.vector.memset(ones_row_f32[:], 1.0)
            ones_row = pool.tile([1, 128], F32R)    # lhsT for broadcast
            nc.vector.tensor_copy(ones_row[:], ones_row_f32[:])

            # ---- PE warm-up while startup DMAs land (HAM un-throttle) ----
            warm_ps = ps_qk.tile([128, 512], F32, tag="qk")
            for _ in range(90):
                nc.tensor.matmul(
                    warm_ps[:, 0:64], zeros16[:], zeros16[:, 0:64],
                    start=True, stop=True,
                )
            exp_dummy = pool.tile([1, 1], BF16)
            nc.scalar.activation(exp_dummy[:], ones_f32[0:1, 0:1], AF.Exp)

            # ---- staging DMAs (first ones split per chunk, block-0 chunk
            # first, so the P phase can start before the rest lands; w2
            # borrows a stream slot and frees it after P) ----
            w2_sb = stream.tile([128, 4, 512], F32R, tag="stream")
            blk0 = stream.tile([128, 4, 512], F32R, tag="stream")
            for dch in range(4):
                nc.sync.dma_start(blk0[:, dch, :], xt[:, 0, dch, :])
                nc.sync.dma_start(w2_sb[:, dch, :], w2[:, dch, :])

            def load_blk(k):
                t = stream.tile([128, 4, 512], F32R, tag="stream")
                # alternate trigger paths: two queues process descriptors
                # in parallel
                eng = nc.sync if k % 2 == 0 else nc.gpsimd
                eng.dma_start(t[:], xt[:, k, :, :])
                return t

            blk01 = {0: blk0, 1: load_blk(1)}

            # ---- P: G^T = (M^T X_mine^T), from own stream blocks ----
            r_sb = pool.tile([128, 4, MY_N], F32R, tag="r")
            for bi in range(2):
                for ech in range(4):
                    g_ps = ps_qk.tile([128, 512], F32, tag="qk")
                    for dch in range(4):
                        nc.tensor.matmul(
                            g_ps[:],
                            w2_sb[:, dch, ts(ech, 128)],
                            blk01[bi][:, dch, :],
                            start=(dch == 0),
                            stop=(dch == 3),
                        )
                    nc.vector.tensor_copy(r_sb[:, ech, ts(bi, 512)], g_ps[:])

            def b1_qk(h, st, mx, preloaded, k_order):
                held = {}
                for k in k_order:
                    blk_t = preloaded.get(k) or load_blk(k)
                    held[k] = blk_t
                    for t in range(4):
                        jt = k * 4 + t
                        s_ps = ps_qk.tile([128, 512], F32, tag="qk")
                        for e in range(4):
                            nc.tensor.matmul(
                                s_ps[:],
                                blk_t[:, e, ts(t, 128)],
                                r_sb[:, e, ts(h, 512)],
                                start=(e == 0),
                                stop=(e == 3),
                            )
                        nc.scalar.copy(st[:, jt, :], s_ps[:])
                        if k == k_order[0] and t == 0:
                            nc.vector.tensor_copy(mx[:], s_ps[:])
                        else:
                            nc.vector.tensor_tensor(
                                mx[:], mx[:], s_ps[:], op=ALU.max
                            )
                return {k: held[k] for k in k_order[-4:]}

            def finalize_max(mx):
                """mx[128,512] -> b_sb[128,512] broadcast of per-i max."""
                mcol = pool.tile([128, 4], F32, tag="mcol")
                for c in range(4):
                    mt_ps = ps_qk.tile([128, 128], F32, tag="qk")
                    nc.tensor.transpose(mt_ps[:], mx[:, ts(c, 128)], ident[:])
                    nc.vector.reduce_max(
                        mcol[:, c : c + 1], mt_ps[:], axis=mybir.AxisListType.X
                    )
                mrow_ps = ps_qk.tile([1, 512], F32, tag="qk")
                for c in range(4):
                    nc.tensor.transpose(
                        mrow_ps[:, ts(c, 128)], mcol[:, c : c + 1], ident[:]
                    )
                mrow = pool.tile([1, 512], F32R, tag="mrow")
                nc.scalar.copy(mrow[:], mrow_ps[:])
                b_ps = ps_qk.tile([128, 512], F32, tag="qk")
                nc.tensor.matmul(b_ps[:], ones_row[:], mrow[:], start=True, stop=True)
                b_sb = pool.tile([128, 512], F32, tag="bsb")
                nc.scalar.copy(b_sb[:], b_ps[:])
                return b_sb

            def b3_exp_and_accum(h, st, b_sb, o_ps, sum_ps):
                # one start=True matmul zeroes the whole sum bank; the per-
                # chunk 1-col sums then accumulate with start=False (a
                # start per chunk would clear the bank and wipe the other
                # columns' first contributions)
                nc.tensor.matmul(
                    sum_ps[:], zeros16[:], ones4[:],
                    start=True, stop=False, skip_group_check=True,
                )
                x_blk = None
                for jt in range(NJT):
                    if jt % 4 == 0:
                        x_blk = xsp.tile([128, 4, 512], BF16, tag="x")
                        nc.gpsimd.dma_start(x_blk[:], x16[:, jt // 4, :, :])
                    x_t = x_blk[:, jt % 4, :]
                    nc.vector.tensor_tensor(
                        st[:, jt, :], st[:, jt, :], b_sb[:], op=ALU.subtract
                    )
                    p_t = work.tile([128, 512], BF16, tag="p")
                    nc.scalar.activation(p_t[:], st[:, jt, :], AF.Exp)
                    for c in range(4):
                        nc.tensor.matmul(
                            o_ps[:, c, :],
                            p_t[:, ts(c, 128)],
                            x_t,
                            start=(jt == 0),
                            stop=(jt == NJT - 1),
                        )
                        nc.tensor.matmul(
                            sum_ps[:, c : c + 1],
                            p_t[:, ts(c, 128)],
                            ones_col[:],
                            start=False,
                            stop=(jt == NJT - 1 and c == 3),
                            skip_group_check=True,
                        )

            def b4_drain(h, o_ps, sum_ps):
                rcol = pool.tile([128, 4], F32, tag="rcol")
                nc.vector.reciprocal(rcol[:], sum_ps[:])
                for c in range(4):
                    o_sb = osbp.tile([128, 512], F32, tag="osb")
                    nc.scalar.activation(
                        o_sb[:], o_ps[:, c, :], AF.Copy,
                        bias=0.0, scale=rcol[:, c : c + 1],
                    )
                    nc.sync.dma_start(o[ts(h * 4 + c, 128), :], o_sb[:])

            pre = blk01
            for h in range(NIH):
                st = big.tile([128, NJT, 512], F32, tag="big")
                mx = pool.tile([128, 512], F32, tag="mx")
                # half 1 walks the blocks in reverse so the four tiles
                # still resident from half 0's tail are reused (no DMA,
                # no entry stall)
                k_order = list(range(NBLK)) if h == 0 else list(range(NBLK - 1, -1, -1))
                pre = b1_qk(h, st, mx, pre, k_order)
                b_sb = finalize_max(mx)
                o_ps = ps_o.tile([128, 4, 512], F32, tag="o")
                sum_ps = ps_sum.tile([128, 4], F32, tag="sum")
                b3_exp_and_accum(h, st, b_sb, o_ps, sum_ps)
                b4_drain(h, o_ps, sum_ps)

    nc.compile()
    return nc


def _get_nc():
    global _NC_CACHE
    if _NC_CACHE is None:
        _NC_CACHE = _build_nc()
    return _NC_CACHE


def kernel(rotation_params, entangle_params, inputs, _trace=False, _trace_kwargs=None):
    X = np.ascontiguousarray(inputs, dtype=np.float32)
    Wq = np.ascontiguousarray(rotation_params, dtype=np.float32)
    Wk = np.ascontiguousarray(entangle_params, dtype=np.float32)
    M = (Wq.astype(np.float64) @ Wk.astype(np.float64).T / np.sqrt(512.0)).astype(
        np.float32
    )
    W2B = np.ascontiguousarray(M.reshape(4, 128, 512).transpose(1, 0, 2))
    XT = np.ascontiguousarray(X.T)
    # blocked layouts: [p, blk, c, j] with 8KiB (f32) / 4KiB (f16) runs/partition
    XTB = np.ascontiguousarray(XT.reshape(4, 128, NBLK, 512).transpose(1, 2, 0, 3))
    X16B = np.ascontiguousarray(
        X.astype(ml_dtypes.bfloat16).reshape(NBLK, 4, 128, 512).transpose(2, 0, 1, 3)
    )

    in_maps = []
    for c in range(NCORES):
        # rotate the j-block order so blocks 0,1 of the stream are this
        # core's own query rows (P phase reuses them for G^T)
        order = [(2 * c + k) % NBLK for k in range(NBLK)]
        in_maps.append(
            {
                "xt": np.ascontiguousarray(XTB[:, order, :, :]),
                "x16": np.ascontiguousarray(X16B[:, order, :, :]),
                "w2": W2B,
            }
        )

    nc = _get_nc()
    kw = {}
    if _trace:
        kw["trace"] = True
        kw.update(_trace_kwargs or {})
    br = run_bass_kernel_spmd(nc, in_maps, core_ids=list(range(NCORES)), **kw)
    out = np.concatenate([r["o"] for r in br.results], axis=0)
    if _trace:
        return out, br
    return out
